# revision 2
# baseline (speedup 1.0000x reference)
# FVSBN kernel for Trainium2: out = x @ (W * tril(-1)).T + b
#   x: [65536, 764] f32, W: [764, 764] f32, b: [764] f32 -> out: [65536, 764] f32
#
# Data-parallel over batch across 8 NeuronCores (8192 rows each). Each core
# computes out^T = WT-tiles.T @ x^T as a block-lower-triangular matmul (output
# tile row nt only needs contraction tiles dt <= nt: 21 of 36 tile pairs).
#
# The matmul runs entirely in fp8 (e4m3) DoubleRow perf mode, which contracts
# TWO 128-deep k-tiles per instruction at ~2.6x the fp16 row rate. Accuracy is
# recovered by residual compensation: x ~ x8 + dx8 and W ~ W8 + dW8 (each an
# e4m3 value plus an e4m3-quantized remainder at the same power-of-2 scale).
# For a pair of k-tiles (d1,d2) feeding output tile nt, three DoubleRow
# matmuls accumulate x8@W8 + dx8@W8 + x8@dW8 (the dropped dx8@dW8 term is
# O(ulp^2)); an unpaired k-tile uses two self-pair matmuls giving the exact
# four-term product. Measured end-to-end rel err ~1.1e-3.
#
#   - moving operand: x-slab SBUF tile [128, 6, 2, 8192] fp8 holding x8 (j=0)
#     and dx8 (j=1) per k-tile; a DoubleRow rhs is a strided 3-D slice
#     [128, 2, 512] selecting either a (d1,d2) pair at fixed j or an
#     (x8,dx8) self-pair at fixed t.
#   - stationary operand: 24 pre-packed units [128, 2, 128] fp8 (pair mains,
#     pair deltas, single duplicated mains/deltas), host-packed.
#   - psum [128 n, 512 b] accumulates all of a row's terms at scale 2^14;
#     eviction is one DVE tensor_scalar: out = psum * 2^-14 + bias.
# Host gathers by transposing each core's out^T back.

import numpy as np

B = 65536
D = 764
NCORES = 8
BPC = B // NCORES  # 8192 rows per core
P = 128
NT = 6  # ceil(764/128)
DP = NT * P  # 768, zero-padded depth
BB = 512  # matmul moving free dim == psum bank width (fp32)

SX = 16.0  # x pre-scale (pow2): x8 = e4m3(16 x)
SW = 1024.0  # W pre-scale (pow2): W8 = e4m3(1024 W^T)
INV_S = 1.0 / (SX * SW)

OUT_DT = "float16"

# Per output-row nt: k-tile pairs and leftover single, and the stationary
# unit schedule. Units are packed host-side in this exact order:
#   pair -> (main: W8[d1]|W8[d2], delta: dW8[d1]|dW8[d2])
#   single -> (smain: W8[s]|W8[s], sdelta: dW8[s]|dW8[s])
def _row_plan():
    plan = []  # per nt: (pairs, single, unit_base)
    units = []  # (nt, kind, d1, d2) kind in {mainpair, deltapair, smain, sdelta}
    for nt in range(NT):
        pairs = [(d, d + 1) for d in range(0, nt, 2)]
        single = nt if (nt + 1) % 2 == 1 else None
        ub = len(units)
        for (d1, d2) in pairs:
            units.append((nt, "mainpair", d1, d2))
            units.append((nt, "deltapair", d1, d2))
        if single is not None:
            units.append((nt, "smain", single, single))
            units.append((nt, "sdelta", single, single))
        plan.append((pairs, single, ub))
    return plan, units


ROW_PLAN, UNITS = _row_plan()
NU = len(UNITS)  # 24


def _np_dt(name):
    import ml_dtypes

    return {
        "float32": np.float32,
        "float16": np.float16,
        "bfloat16": ml_dtypes.bfloat16,
        "float8e4": ml_dtypes.float8_e4m3,
    }[name]


def _build(bpc, reps=1, ablate_x=False, ablate_out=False, xc=2048):
    import concourse.mybir as mybir
    from concourse import bacc
    from concourse.tile import TileContext

    f8 = mybir.dt.float8e4
    f32 = mybir.dt.float32
    out_dt = getattr(mybir.dt, OUT_DT)
    DR = mybir.MatmulPerfMode.DoubleRow

    nc = bacc.Bacc("TRN2", target_bir_lowering=False, debug=False)
    xq = nc.dram_tensor("xq", [2 * DP, bpc], f8, kind="ExternalInput")
    wq = nc.dram_tensor("wq", [P, NU * 2 * P], f8, kind="ExternalInput")
    bias = nc.dram_tensor("bias", [P, NT], f32, kind="ExternalInput")
    outT = nc.dram_tensor("outt", [DP, bpc], out_dt, kind="ExternalOutput")

    HB = bpc // 2  # batch half per x-load/compute pipeline stage
    nhb = HB // BB
    GRP = min(4, nhb)  # psum banks sharing one weight load burst

    with TileContext(nc) as tc:
        with (
            tc.tile_pool(name="wpool", bufs=1) as wpool,
            tc.tile_pool(name="bpool", bufs=1) as bpool,
            tc.tile_pool(name="xpool", bufs=1) as xpool,
            tc.tile_pool(name="opool", bufs=3) as opool,
            tc.tile_pool(name="pspool", bufs=8, space="PSUM") as pspool,
        ):
            w_sb = wpool.tile([P, NU, 2, P], f8)
            nc.sync.dma_start(out=w_sb, in_=wq.ap())
            bias_sb = bpool.tile([P, NT], f32)
            nc.sync.dma_start(out=bias_sb, in_=bias.ap())

            # x-slab: [p, t, j, b] with j=0 -> x8, j=1 -> dx8
            x_sb = xpool.tile([P, NT, 2, bpc], f8, tag="xslab", name="xslab")

            XC = min(xc, HB)

            def load_x():
                for half in range(2):
                    for t in range(NT):
                        for j in range(2):
                            r0 = (2 * t + j) * P
                            for c0 in range(half * HB, (half + 1) * HB, XC):
                                nc.sync.dma_start(
                                    out=x_sb[:, t, j, c0 : c0 + XC],
                                    in_=xq.ap()[r0 : r0 + P, c0 : c0 + XC],
                                )

            def body():
                if not ablate_x:
                    load_x()
                for half in range(2):
                    for nt in range(NT):
                        pairs, single, ub = ROW_PLAN[nt]
                        # (unit_idx, moving_kind, d) schedule; consecutive
                        # same-unit mms keep the PE weights loaded.
                        mms = []
                        u = ub
                        for (d1, d2) in pairs:
                            mms.append((u, "x", d1))  # x8 pair @ mains
                            mms.append((u, "dx", d1))  # dx8 pair @ mains
                            mms.append((u + 1, "x", d1))  # x8 pair @ deltas
                            u += 2
                        if single is not None:
                            mms.append((u, "s", single))
                            mms.append((u + 1, "s", single))
                        o_nt = opool.tile([P, HB], out_dt)
                        for grp in range(nhb // GRP):
                            pss = [
                                pspool.tile([P, BB], f32, name="ps")
                                for _ in range(GRP)
                            ]
                            for mi, (ui, mk, d) in enumerate(mms):
                                for g4 in range(GRP):
                                    c0 = half * HB + (grp * GRP + g4) * BB
                                    if mk == "x":
                                        mv = x_sb[:, d : d + 2, 0, c0 : c0 + BB]
                                    elif mk == "dx":
                                        mv = x_sb[:, d : d + 2, 1, c0 : c0 + BB]
                                    else:
                                        mv = x_sb[:, d, :, c0 : c0 + BB]
                                    nc.tensor.matmul(
                                        pss[g4],
                                        w_sb[:, ui],
                                        mv,
                                        start=(mi == 0),
                                        stop=(mi == len(mms) - 1),
                                        perf_mode=DR,
                                    )
                            for g4 in range(GRP):
                                bg = grp * GRP + g4
                                nc.vector.tensor_scalar(
                                    out=o_nt[:, bg * BB : (bg + 1) * BB],
                                    in0=pss[g4],
                                    scalar1=INV_S,
                                    scalar2=bias_sb[:, nt : nt + 1],
                                    op0=mybir.AluOpType.mult,
                                    op1=mybir.AluOpType.add,
                                )
                        # out-writes ride the scalar-engine HWDGE ring so they
                        # don't queue behind x-loads on the sync ring
                        if ablate_out:
                            nc.scalar.dma_start(
                                out=outT.ap()[nt * P : (nt + 1) * P, 0:8],
                                in_=o_nt[:, 0:8],
                            )
                        else:
                            nc.scalar.dma_start(
                                out=outT.ap()[
                                    nt * P : (nt + 1) * P, half * HB : (half + 1) * HB
                                ],
                                in_=o_nt,
                            )

            if ablate_x:
                load_x()
            if reps == 1:
                body()
            else:
                with tc.For_i(0, reps, 1, hint_engines=(mybir.EngineType.PE,)):
                    body()
    nc.compile()
    _dedup_ldweights(nc, mybir)
    return nc


def _dedup_ldweights(nc, mybir):
    """Remove back-to-back redundant LDWEIGHTS: within a basic block, a
    Ldweights whose weight AP matches the previous PE weight load (with no
    intervening write to that SBUF region and no semaphores attached) leaves
    the PE array state unchanged and can be dropped."""
    n_removed = 0
    for blk in nc.m.functions[0].blocks:
        il = blk.instructions
        last_sig = None
        to_remove = []
        for inst in il:
            if isinstance(inst, mybir.InstLdweights):
                a = inst.ins[0]
                sig = (
                    a.memref,
                    a.offset,
                    str(a.ap),
                    str(a.dtype),
                    bool(inst.is_transpose),
                )
                if sig == last_sig and not inst.has_wait() and not inst.has_update():
                    to_remove.append(inst)
                else:
                    last_sig = sig
            elif isinstance(inst, mybir.InstMatmult):
                continue
            else:
                if last_sig is not None:
                    try:
                        outs = inst.outs
                    except AttributeError:
                        outs = []
                    for o in outs or []:
                        if getattr(o, "memref", None) == last_sig[0]:
                            last_sig = None
                            break
        for inst in to_remove:
            il.remove(inst)
        n_removed += len(to_remove)
    return n_removed


def _quant_split(a, scale, e4):
    """a -> (hi, lo) e4m3 arrays with hi+lo ~ scale*a (both at the same
    scale so their matmul products accumulate in one psum)."""
    s = (scale * a).astype(np.float32)
    hi = s.astype(e4)
    lo = (s - np.asarray(hi, np.float32)).astype(e4)
    return hi, lo


def _prep_shared(W, b):
    import ml_dtypes

    e4 = ml_dtypes.float8_e4m3
    Wm = (np.asarray(W, np.float32) * np.tril(np.ones((D, D), np.float32), k=-1))
    WT = np.zeros((DP, DP), np.float32)
    WT[:D, :D] = Wm.T  # WT[d, n] = Wm[n, d]
    W8, dW8 = _quant_split(WT, SW, e4)
    w_packed = np.zeros((P, NU, 2, P), e4)
    for ui, (nt, kind, d1, d2) in enumerate(UNITS):
        src = W8 if kind in ("mainpair", "smain") else dW8
        w_packed[:, ui, 0, :] = src[d1 * P : (d1 + 1) * P, nt * P : (nt + 1) * P]
        w_packed[:, ui, 1, :] = src[d2 * P : (d2 + 1) * P, nt * P : (nt + 1) * P]
    w_packed = np.ascontiguousarray(w_packed.reshape(P, NU * 2 * P))
    bias_pad = np.zeros(DP, np.float32)
    bias_pad[:D] = np.asarray(b, np.float32)
    bias_t = np.ascontiguousarray(bias_pad.reshape(NT, P).T)  # [p, t]
    return w_packed, bias_t


def _prep_x_core(xs):
    """xs: [bpc, D] f32 slice -> xq [2*DP, bpc] fp8 (x8/dx8 interleaved by
    k-tile: rows (2t+j)*128 .. +128)."""
    import ml_dtypes

    e4 = ml_dtypes.float8_e4m3
    bpc = xs.shape[0]
    xT = np.zeros((DP, bpc), np.float32)
    xT[:D] = np.asarray(xs, np.float32).T
    x8, dx8 = _quant_split(xT, SX, e4)
    xqc = np.zeros((2 * DP, bpc), e4)
    for t in range(NT):
        xqc[(2 * t) * P : (2 * t + 1) * P] = x8[t * P : (t + 1) * P]
        xqc[(2 * t + 1) * P : (2 * t + 2) * P] = dx8[t * P : (t + 1) * P]
    return xqc


def kernel(x, W, b):
    from concourse.bass_utils import run_bass_kernel_spmd

    nc = _build(BPC)
    w_packed, bias_t = _prep_shared(W, b)

    in_maps = []
    for c in range(NCORES):
        xqc = _prep_x_core(x[c * BPC : (c + 1) * BPC])
        in_maps.append({"xq": xqc, "wq": w_packed, "bias": bias_t})

    res = run_bass_kernel_spmd(nc, in_maps, core_ids=list(range(NCORES)))

    out = np.empty((B, D), np.float32)
    for c in range(NCORES):
        out[c * BPC : (c + 1) * BPC] = (
            res.results[c]["outt"][:D].astype(np.float32).T
        )
    return out
